# revision 17
# baseline (speedup 1.0000x reference)
"""Trainium2 Bass kernel for nn_MILoss (Parzen-window mutual-information loss).

Contract: kernel(**inputs) takes the FULL inputs (fix_img [2,1,64,128,128] f32,
reg_img same, rand_index [2,200000] int64) and returns the FULL output (scalar
f32), sharding internally across 8 NeuronCores.

Per core: core g handles sample b = g//4 and a 50k block of the 200k sampled
indices (host gathers the (x,y) voxel pairs into per-core x/y planes). Each
sample contributes relu(exp(-((dx-mu_i)^2+(dy-mu_j)^2)/(2s^2)) - e^-0.25) to a
2x2 patch of the 40x40 joint histogram; the scatter is a one-hot matmul on the
TensorEngine accumulating a [40,160] PSUM (4 corner blocks). Broadcasts of the
per-sample scalars along the 40-bin axis are done as fp32 *pair* copies on the
Scalar engine (two adjacent samples' fp16 values ride in one fp32), which
halves ACT traffic and makes the is_equal one-hots and the weight multiply
dense 2x-mode DVE ops on the pair-interleaved layout; the matmul reads the
de-interleaved lhsT/rhs through stride-2 access patterns (free on the PE).
Bin values are biased by +64 and weights clamped to the fp16 min normal so
every packed fp32 stays a normal number through the ACT copy. Each core DMAs
its partial [40,160] block histogram out; the host sums the 8 partials (the
unshard step) and applies the final MI formula on the 40x40 joint histogram.
"""

import math
from contextlib import ExitStack

import numpy as np

import concourse.bass as bass
import concourse.bacc as bacc
import concourse.mybir as mybir
import concourse.tile as tile
from concourse.bass_utils import run_bass_kernel_spmd

AF = mybir.ActivationFunctionType
ALU = mybir.AluOpType
DT = mybir.dt

NB = 40
CREL = math.exp(-0.25)
SQ2 = 0.7071067811865476
EPS16 = 6.104e-5  # fp16 min normal: keeps packed fp32 pairs normal
BIAS = 64.0  # bin-value bias: keeps packed fp32 pairs normal

N_VOX = 1 * 64 * 128 * 128  # 1048576
N_IDX = 200000
N_CORES = 8
CORES_PER_B = 4
N_REAL = N_IDX // CORES_PER_B  # 50000 per core
F = 392  # 128*392 = 50176 sample slots (176 padded)

# chunk sizes (must each be even); two small lead-in chunks shorten the
# dependency chain to the first matmul
CHUNKS = [28, 28] + [56] * 6
# small-stage slice widths; chunk i reads slice SLICE_OF[i] at offset OFF[i]
SLICES = [56, 112, 112, 112]
_slice_starts = np.cumsum([0] + SLICES)
_chunk_starts = np.cumsum([0] + CHUNKS)
SLICE_OF = []
OFF = []
for i, c0 in enumerate(_chunk_starts[:-1]):
    si = int(np.searchsorted(_slice_starts, c0, side="right") - 1)
    SLICE_OF.append(si)
    OFF.append(int(c0 - _slice_starts[si]))
assert _chunk_starts[-1] == F == _slice_starts[-1]


def build_mi_kernel(robust_floor=False):
    """robust_floor=True uses the rounding-mode-agnostic floor (needed for
    CoreSim, which truncates fp32->int; HW rounds to nearest)."""
    nc = bacc.Bacc(None)
    # x/y: per-core pre-gathered sample planes; padding slots hold 9.0 which
    # lands on bin 359+BIAS and never matches the one-hot.
    x_d = nc.declare_dram_parameter("x", [128, F], DT.float32, isOutput=False)
    y_d = nc.declare_dram_parameter("y", [128, F], DT.float32, isOutput=False)
    out_d = nc.declare_dram_parameter("out", [NB, 4 * NB], DT.float32, isOutput=True)

    with tile.TileContext(nc) as tc, ExitStack() as ctx:
        pools = {}

        def P(name, bufs, space="SBUF"):
            if name not in pools:
                pools[name] = ctx.enter_context(
                    tc.tile_pool(name=name, bufs=bufs, space=space)
                )
            return pools[name]

        cst = P("cst", 1)
        iota_i = cst.tile([128, 2 * NB], DT.int32, tag="iota_i")
        nc.gpsimd.iota(
            iota_i[:], pattern=[[1, NB], [0, 2]], base=0, channel_multiplier=0
        )
        iota_f = cst.tile([128, 2 * NB], DT.float32, tag="iota_f")
        nc.vector.tensor_copy(iota_f[:], iota_i[:])
        iota_d = cst.tile([128, 2 * NB], DT.float16, tag="iota_d")
        nc.vector.tensor_scalar(iota_d[:], iota_f[:], 1.0, BIAS, ALU.mult, ALU.add)
        def bconst(tag, val):
            t = cst.tile([128, 1], DT.float32, tag=tag, name=tag)
            nc.vector.memset(t[:], val)
            return t

        # Square-activation biases: z = dz (robust path, u = 40x-0.5) or
        # z = dz + 0.5 (rni path, u2 = 40x-1)
        if robust_floor:
            U_BIAS = -0.5
            b_sq0 = None
            b_sq1 = bconst("b_sq1", -SQ2)
        else:
            U_BIAS = -1.0
            b_sq0 = bconst("b_sq0", 0.5 * SQ2)
            b_sq1 = bconst("b_sq1", -0.5 * SQ2)

        sm = P("small", 1)
        x_sb = sm.tile([128, F], DT.float32, tag="x")
        nc.sync.dma_start(x_sb[:], x_d[:])
        y_sb = sm.tile([128, F], DT.float32, tag="y")
        nc.sync.dma_start(y_sb[:], y_d[:])

        # small stage, sliced along F with per-slice tiles so chunk 0 isn't
        # gated on the full-F chain. Outputs per slice: rc16 [128,2,SL]
        # (biased bin values, r then c) and w4 [128,4,SL] (corner weights).
        rcw = []
        smt = P("smt", 2)
        for si, SL in enumerate(SLICES):
            s0 = int(_slice_starts[si])
            n = SL
            rc16 = sm.tile([128, 2, SL], DT.float16, tag=f"rc16_{si}", name=f"rc16_{si}")
            w4 = sm.tile([128, 4, SL], DT.float16, tag=f"w4_{si}", name=f"w4_{si}")
            res = {}
            for xi, (ax, src) in enumerate((("r", x_sb), ("c", y_sb))):
                # u on ACT: u = 40x + b_u (b_u = -0.5 robust, -1.0 rni)
                u = smt.tile([128, SL], DT.float32, tag=f"u{ax}", name=f"u{ax}")
                nc.scalar.activation(
                    u[:, :n], src[:, s0 : s0 + n], AF.Copy, scale=40.0, bias=U_BIAS
                )
                ri = smt.tile([128, SL], DT.int32, tag=f"ri{ax}", name=f"ri{ax}")
                nc.vector.tensor_copy(ri[:, :n], u[:, :n])
                rf0 = smt.tile([128, SL], DT.float32, tag=f"rf0{ax}", name=f"rf0{ax}")
                nc.vector.tensor_copy(rf0[:, :n], ri[:, :n])
                if robust_floor:
                    # rounding-mode-agnostic floor: correct round-up (d<0)
                    d = smt.tile([128, SL], DT.float32, tag=f"d{ax}", name=f"d{ax}")
                    nc.vector.tensor_sub(d[:, :n], u[:, :n], rf0[:, :n])
                    lt = smt.tile([128, SL], DT.float32, tag=f"lt{ax}", name=f"lt{ax}")
                    nc.vector.tensor_single_scalar(
                        lt[:, :n], d[:, :n], 0.0, ALU.is_lt
                    )
                    rfm = smt.tile(
                        [128, SL], DT.float32, tag=f"rfm{ax}", name=f"rfm{ax}"
                    )
                    nc.vector.tensor_sub(rfm[:, :n], rf0[:, :n], lt[:, :n])
                else:
                    # HW fp32->int copy rounds to nearest: rni(40x-1) is
                    # floor(40x-0.5) away from exact .5 ties
                    rfm = rf0
                rf = smt.tile([128, SL], DT.float32, tag=f"rf{ax}", name=f"rf{ax}")
                nc.vector.tensor_scalar_max(rf[:, :n], rfm[:, :n], 0.0)
                dz = smt.tile([128, SL], DT.float32, tag=f"dz{ax}", name=f"dz{ax}")
                nc.vector.tensor_sub(dz[:, :n], u[:, :n], rf[:, :n])
                nc.scalar.activation(
                    rc16[:, xi, :n], rf[:, :n], AF.Copy, scale=1.0, bias=BIAS
                )
                # e0 = exp(-z^2/2), e1 = exp(-(z-1)^2/2); z = dz (+0.5 on rni)
                sq0 = smt.tile([128, SL], DT.float32, tag=f"sq0{ax}", name=f"sq0{ax}")
                if b_sq0 is None:
                    nc.scalar.activation(sq0[:, :n], dz[:, :n], AF.Square, scale=SQ2)
                else:
                    nc.scalar.activation(
                        sq0[:, :n], dz[:, :n], AF.Square, scale=SQ2, bias=b_sq0[:]
                    )
                sq1 = smt.tile([128, SL], DT.float32, tag=f"sq1{ax}", name=f"sq1{ax}")
                nc.scalar.activation(
                    sq1[:, :n], dz[:, :n], AF.Square, scale=SQ2, bias=b_sq1[:]
                )
                e0 = smt.tile([128, SL], DT.float16, tag=f"e0{ax}", name=f"e0{ax}")
                nc.scalar.activation(e0[:, :n], sq0[:, :n], AF.Exp, scale=-1.0)
                e1 = smt.tile([128, SL], DT.float16, tag=f"e1{ax}", name=f"e1{ax}")
                nc.scalar.activation(e1[:, :n], sq1[:, :n], AF.Exp, scale=-1.0)
                res[ax] = (e0, e1)
            p0, p1 = res["r"]
            q0, q1 = res["c"]
            for qi, (pa, qb) in enumerate(((p0, q0), (p0, q1), (p1, q0), (p1, q1))):
                v = smt.tile([128, SL], DT.float16, tag=f"v{qi}", name=f"v{qi}")
                nc.vector.tensor_mul(v[:, :n], pa[:, :n], qb[:, :n])
                # clamp to fp16 min normal (keeps packed fp32 normal; adds
                # only ~1e-4 relative mass to the histogram)
                nc.vector.tensor_scalar(
                    w4[:, qi, :n], v[:, :n], CREL, EPS16, ALU.subtract, ALU.max
                )
            rcw.append((rc16, w4))

        psum = P("psum", 1, space="PSUM")
        hist_ps = psum.tile([NB, 4 * NB], DT.float32, tag="hist")

        big = P("big", 3)
        mm_i = 0
        for t, K in enumerate(CHUNKS):
            K2 = K // 2
            rc16, w4 = rcw[SLICE_OF[t]]
            o2 = OFF[t] // 2  # fp32-pair offset within the slice

            # ACT: fp32-pair broadcasts (each fp32 carries two samples' fp16)
            rc_b = big.tile([128, 2, K2, NB], DT.float32, tag="rc_b")
            rc32 = rc16[:].bitcast(DT.float32)[:, :, o2 : o2 + K2]
            nc.scalar.activation(
                rc_b[:], rc32.unsqueeze(3).broadcast_to([128, 2, K2, NB]), AF.Copy
            )
            w_b = big.tile([128, 4, K2, NB], DT.float32, tag="w_b")
            w32 = w4[:].bitcast(DT.float32)[:, :, o2 : o2 + K2]
            nc.scalar.activation(
                w_b[:], w32.unsqueeze(3).broadcast_to([128, 4, K2, NB]), AF.Copy
            )

            # DVE: pair-interleaved one-hots (dense 2x) + weight multiply
            AC = big.tile([128, 2, K2, 2 * NB], DT.float16, tag="AC")
            nc.vector.tensor_tensor(
                AC[:],
                iota_d[:]
                .unsqueeze(1)
                .unsqueeze(1)
                .broadcast_to([128, 2, K2, 2 * NB]),
                rc_b[:].bitcast(DT.float16),
                ALU.is_equal,
            )
            R = big.tile([128, K2, 4, 2 * NB], DT.float16, tag="R")
            nc.vector.tensor_tensor(
                R[:],
                AC[:, 1, :, :].unsqueeze(2).broadcast_to([128, K2, 4, 2 * NB]),
                w_b[:].bitcast(DT.float16).rearrange("p q t j -> p t q j"),
                ALU.mult,
            )

            # PE: de-interleave through stride-2 APs
            for t2 in range(K2):
                av = AC[:, 0, t2, :].rearrange("p (j s) -> p s j", s=2)
                rv = R[:, t2, :, :].rearrange("p q (j s) -> p s (q j)", s=2)
                for s in range(2):
                    nc.tensor.matmul(
                        hist_ps[:],
                        lhsT=av[:, s, :],
                        rhs=rv[:, s, :],
                        start=(mm_i == 0),
                        stop=(mm_i == F - 1),
                    )
                    mm_i += 1

        fin = P("fin", 1)
        hist_sb = fin.tile([NB, 4 * NB], DT.float32, tag="hist_sb")
        nc.scalar.copy(hist_sb[:], hist_ps[:])
        nc.sync.dma_start(out_d[:, :], hist_sb[:])

    nc.finalize()
    return nc


def make_in_maps(fix_img, reg_img, rand_index):
    xf = np.asarray(fix_img, np.float32).reshape(2, -1)
    yf = np.asarray(reg_img, np.float32).reshape(2, -1)
    ridx = np.asarray(rand_index)
    in_maps = []
    pad = 128 * F - N_REAL
    for g in range(N_CORES):
        b, q = g // CORES_PER_B, g % CORES_PER_B
        ids = ridx[b, q * N_REAL : (q + 1) * N_REAL]
        xs = np.concatenate([xf[b][ids], np.full(pad, 9.0, np.float32)])
        ys = np.concatenate([yf[b][ids], np.full(pad, 9.0, np.float32)])
        in_maps.append(
            {
                "x": np.ascontiguousarray(xs.reshape(128, F)),
                "y": np.ascontiguousarray(ys.reshape(128, F)),
            }
        )
    return in_maps


def combine_host(block_hists):
    """block_hists: list of 8 [40,160] partial block histograms -> loss."""
    Hb = np.zeros((2, NB, NB), np.float64)
    for g, bh in enumerate(block_hists):
        B = np.asarray(bh, np.float64).reshape(NB, 4, NB)
        full = np.zeros((NB + 1, NB + 1), np.float64)
        for qi, (a, b) in enumerate(((0, 0), (0, 1), (1, 0), (1, 1))):
            full[a : NB + a, b : NB + b] += B[:, qi, :]
        Hb[g // CORES_PER_B] += full[:NB, :NB]
    losses = []
    for b in range(2):
        H = Hb[b]
        pxy = H / H.sum()
        px = pxy.sum(axis=1, keepdims=True)
        py = pxy.sum(axis=0, keepdims=True)
        losses.append(
            -np.sum(pxy * np.log(pxy + 1e-9) - pxy * np.log(px * py + 1e-9))
        )
    return np.float32(sum(losses) / 2.0)


_NC_CACHE = {}


def _get_nc():
    if "nc" not in _NC_CACHE:
        _NC_CACHE["nc"] = build_mi_kernel()
    return _NC_CACHE["nc"]


def run_on_hw(fix_img, reg_img, rand_index, trace=False):
    nc = _get_nc()
    in_maps = make_in_maps(fix_img, reg_img, rand_index)
    res = run_bass_kernel_spmd(nc, in_maps, core_ids=list(range(N_CORES)), trace=trace)
    hists = [np.asarray(res.results[g]["out"], np.float32) for g in range(N_CORES)]
    return combine_host(hists), res


def kernel(fix_img, reg_img, rand_index):
    val, _ = run_on_hw(fix_img, reg_img, rand_index, trace=False)
    return np.asarray(val, dtype=np.float32)
